# revision 15
# baseline (speedup 1.0000x reference)
"""Cayley soliton propagator — fused single-pass Trainium2 Bass kernel.

Math: the reference runs 20 PCG iterations on (I + i*k*H) x = (I - i*k*H)
rot(psi) per (batch,token) system, where H is a fixed circulant stencil along
D.  H diagonalizes in the DFT basis with eigenvalues lam_f, so in frequency
space A = 1 + i*k*lam_f is DIAGONAL and every CG iterate is elementwise over
frequencies, with only per-system reductions (dot products over f).  The whole
pipeline therefore fuses into ONE device kernel:

  elementwise phase rotation -> forward modified DFT (PE matmul, the
  (1 - i*k*lam) rhs factor folded into the matrix) -> 20 CG iterations in
  frequency space (vector/pool/scalar engines, diagonal ops + per-system
  accumulations) -> inverse DFT (PE) -> interleaved [.., D, 2] output.

No host round-trip, no collectives.  I/O travels as fp16 (the axon tunnel is
~70 MB/s, so halving bytes halves wall time); all device compute is f32.

Sharding: data-parallel over the flattened system axis N=B*S across 8 cores.
"""

import sys

for _p in ("/opt/trn_rl_repo",):
    if _p not in sys.path:
        sys.path.insert(0, _p)

import numpy as np
import concourse.bass as bass
import concourse.tile as tile
from concourse import bacc, mybir
from concourse.bass_utils import run_bass_kernel_spmd
from concourse.masks import make_identity

f32 = mybir.dt.float32
f16 = mybir.dt.float16
OP = mybir.AluOpType
AF = mybir.ActivationFunctionType

# ---- problem constants (hardcoded per contract) ----
B, S, D = 4, 4096, 512
N = B * S                       # 16384 systems
NCORES = 8
NSYS = N // NCORES              # 2048 systems per core
NTILE = NSYS // 128             # 16 sys-tiles of 128 per core
DT = 0.1
KAP = DT / 2.0                  # 0.05
NIT = 20
NUM_SCALES, BASE_SPARSITY = 3, 5
OFFSETS = [(2 ** s) * j for s in range(NUM_SCALES) for j in range(1, BASE_SPARSITY + 1)]
KCH = 4                         # 512/128 chunks


def _host_const(edge_weights, alpha):
    """Constant tensors for the kernel (computed in fp64, shipped as f32)."""
    w = edge_weights.reshape(-1).astype(np.float64)
    f = np.arange(D)
    deg = 2.0 * w.sum()
    lam = deg - sum(w[k] * 2.0 * np.cos(2 * np.pi * OFFSETS[k] * f / D)
                    for k in range(len(w)))
    dk = KAP * deg
    inv_s2 = 1.0 / (1.0 + dk * dk)

    dmat = np.outer(f, f)
    F = np.exp(-2j * np.pi * dmat / D)              # F[f, d], symmetric
    Fp = (1.0 - 1j * KAP * lam)[:, None] * F        # rhs factor folded in
    PT = np.ascontiguousarray(Fp.real.T)            # [d, f] rhs for fwd matmul
    QT = np.ascontiguousarray(Fp.imag.T)
    QTn = np.ascontiguousarray(-Fp.imag.T)
    G = np.exp(2j * np.pi * dmat / D) / D           # inverse DFT [f, d], symmetric
    GR = np.ascontiguousarray(G.real)
    GI = np.ascontiguousarray(G.imag)
    GIn = np.ascontiguousarray(-G.imag)
    kl = (KAP * lam).reshape(1, D)
    aabs = np.abs(alpha.astype(np.float64)).reshape(1, D)
    cc = np.zeros((1, D))
    cc[0, :4] = [inv_s2, -dk * inv_s2, dk * inv_s2, inv_s2 / D]
    c = dict(PT=PT, QT=QT, QTn=QTn, GR=GR, GI=GI, GIn=GIn,
             kl=kl, aabs=aabs, cc=cc)
    return {k: np.ascontiguousarray(v.astype(np.float32)) for k, v in c.items()}


CONST_NAMES = ("PT", "QT", "QTn", "GR", "GI", "GIn", "kl", "aabs", "cc")


# --------------------------------------------------------------- fused kernel
def _build_fused(ntile=NTILE):
    nsys = ntile * 128
    nc = bacc.Bacc()
    pr_d = nc.declare_dram_parameter("pr16", [nsys, D], f16, isOutput=False)
    pi_d = nc.declare_dram_parameter("pi16", [nsys, D], f16, isOutput=False)
    mat_d = {m: nc.declare_dram_parameter(m, [D, D], f32, isOutput=False)
             for m in ("PT", "QT", "QTn", "GR", "GI", "GIn")}
    kl_d = nc.declare_dram_parameter("kl", [1, D], f32, isOutput=False)
    aa_d = nc.declare_dram_parameter("aabs", [1, D], f32, isOutput=False)
    cc_d = nc.declare_dram_parameter("cc", [1, D], f32, isOutput=False)
    x_d = nc.declare_dram_parameter("xout", [nsys, 2 * D], f16, isOutput=True)

    with tile.TileContext(nc) as tc:
        with tc.tile_pool(name="singles", bufs=1) as singles, \
             tc.tile_pool(name="io", bufs=3) as io, \
             tc.tile_pool(name="tmp", bufs=2) as tmp, \
             tc.tile_pool(name="cols", bufs=2) as colsp, \
             tc.tile_pool(name="trT", bufs=2) as trTp, \
             tc.tile_pool(name="bh", bufs=2) as bhp, \
             tc.tile_pool(name="cg", bufs=2) as cgp, \
             tc.tile_pool(name="sc", bufs=2) as scp, \
             tc.tile_pool(name="xt", bufs=2) as xtp, \
             tc.tile_pool(name="outp", bufs=3) as outp, \
             tc.tile_pool(name="pst", bufs=2, space="PSUM") as pst, \
             tc.tile_pool(name="psf", bufs=1, space="PSUM") as psf, \
             tc.tile_pool(name="psx", bufs=1, space="PSUM") as psx:

            # ---- constants into SBUF ----
            mats = {}
            for m in ("PT", "QT", "QTn", "GR", "GI", "GIn"):
                t = singles.tile([128, KCH * D], f32, name=m)
                for k in range(KCH):
                    nc.sync.dma_start(t[:, k * D:(k + 1) * D],
                                      mat_d[m][k * 128:(k + 1) * 128, :])
                mats[m] = t
            KL = singles.tile([128, D], f32, name="KL")
            nc.gpsimd.dma_start(out=KL[:], in_=kl_d[:].to_broadcast([128, D]))
            aab = singles.tile([128, D], f32, name="aab")
            nc.gpsimd.dma_start(out=aab[:], in_=aa_d[:].to_broadcast([128, D]))
            CC = singles.tile([128, D], f32, name="CC")
            nc.gpsimd.dma_start(out=CC[:], in_=cc_d[:].to_broadcast([128, D]))
            ident = singles.tile([128, 128], f32, name="ident")
            make_identity(nc, ident[:])
            nhalfpi = singles.tile([128, 1], f32, name="nhalfpi")
            nc.vector.memset(nhalfpi[:], float(-np.pi / 2))
            c_is2 = CC[:, 0:1]       # inv_s2
            c_nds = CC[:, 1:2]       # -dk*inv_s2
            c_pds = CC[:, 2:3]       # +dk*inv_s2
            c_isd = CC[:, 3:4]       # inv_s2/D

            for t0 in range(ntile):
                rows = slice(t0 * 128, (t0 + 1) * 128)

                # ======== stage A: load + phase rotation ========
                pr16 = io.tile([128, D], f16, tag="pr16")
                pi16 = io.tile([128, D], f16, tag="pi16")
                nc.sync.dma_start(pr16[:], pr_d[rows, :])
                nc.sync.dma_start(pi16[:], pi_d[rows, :])
                prt = tmp.tile([128, D], f32, tag="prt")
                pit = tmp.tile([128, D], f32, tag="pit")
                nc.scalar.copy(prt[:], pr16[:])
                nc.vector.tensor_copy(pit[:], pi16[:])

                cols = colsp.tile([128, 16], f32, tag="cols")
                ta = tmp.tile([128, D], f32, tag="ta")
                tb = tmp.tile([128, D], f32, tag="tb")
                nc.vector.scalar_tensor_tensor(
                    out=ta[:], in0=prt[:], scalar=1.0, in1=prt[:],
                    op0=OP.mult, op1=OP.mult, accum_out=cols[:, 0:1])
                nc.vector.scalar_tensor_tensor(
                    out=tb[:], in0=pit[:], scalar=1.0, in1=pit[:],
                    op0=OP.mult, op1=OP.mult, accum_out=cols[:, 1:2])
                ir = tmp.tile([128, D], f32, tag="ir")
                nc.gpsimd.tensor_tensor(out=ir[:], in0=ta[:], in1=tb[:], op=OP.add)
                # norm_in = c0+c1 ; rm = 1/max(norm_in/D, 1e-6) ; nrm = -rm
                nc.vector.tensor_tensor(out=cols[:, 2:3], in0=cols[:, 0:1],
                                        in1=cols[:, 1:2], op=OP.add)
                nc.vector.tensor_scalar(out=cols[:, 3:4], in0=cols[:, 2:3],
                                        scalar1=1.0 / D, scalar2=1e-6,
                                        op0=OP.mult, op1=OP.max)
                nc.vector.reciprocal(out=cols[:, 4:5], in_=cols[:, 3:4])
                nc.vector.tensor_scalar(out=cols[:, 5:6], in0=cols[:, 4:5],
                                        scalar1=-1.0, scalar2=None, op0=OP.mult)
                # u = exp(-ir*rm); half-angle: cos_p = 1-2*sin^2(pi*u-pi/2) ...
                u = tmp.tile([128, D], f32, tag="u")
                nc.scalar.activation(out=u[:], in_=ir[:], func=AF.Exp,
                                     bias=0.0, scale=cols[:, 5:6])
                shalf = tmp.tile([128, D], f32, tag="ta")
                nc.scalar.activation(out=shalf[:], in_=u[:], func=AF.Sin,
                                     bias=nhalfpi[:], scale=float(np.pi))
                chalf = tmp.tile([128, D], f32, tag="tb")
                nc.scalar.activation(out=chalf[:], in_=u[:], func=AF.Sin,
                                     bias=0.0, scale=float(np.pi))
                q1 = tmp.tile([128, D], f32, tag="u")
                nc.vector.tensor_tensor(out=q1[:], in0=shalf[:], in1=shalf[:], op=OP.mult)
                cp = tmp.tile([128, D], f32, tag="cp")
                nc.vector.tensor_scalar(out=cp[:], in0=q1[:], scalar1=-2.0,
                                        scalar2=1.0, op0=OP.mult, op1=OP.add)
                q2 = tmp.tile([128, D], f32, tag="q2")
                nc.gpsimd.tensor_tensor(out=q2[:], in0=shalf[:], in1=chalf[:], op=OP.mult)
                sp = tmp.tile([128, D], f32, tag="sp")
                nc.vector.tensor_scalar(out=sp[:], in0=q2[:], scalar1=-2.0,
                                        scalar2=None, op0=OP.mult)
                # env = min(1 + aabs*(ir*rm)^2, 10) ; renv = 1/env
                tsq = tmp.tile([128, D], f32, tag="ta")
                nc.scalar.activation(out=tsq[:], in_=ir[:], func=AF.Square,
                                     bias=0.0, scale=cols[:, 4:5])
                env = tmp.tile([128, D], f32, tag="tb")
                nc.vector.scalar_tensor_tensor(
                    out=env[:], in0=tsq[:], scalar=1.0, in1=aab[:],
                    op0=OP.mult, op1=OP.mult)
                nc.vector.tensor_scalar(out=env[:], in0=env[:],
                                        scalar1=1.0, scalar2=10.0,
                                        op0=OP.add, op1=OP.min)
                renv = tmp.tile([128, D], f32, tag="renv")
                nc.vector.reciprocal_approx_fast(out=renv[:], in_=env[:])
                renv2 = tmp.tile([128, D], f32, tag="ta")
                nc.scalar.activation(out=renv2[:], in_=renv[:], func=AF.Square)
                # norm_rot = sum(ir * renv^2) (|rot|^2 == ir pointwise)
                scr = tmp.tile([128, D], f32, tag="tb")
                nc.vector.scalar_tensor_tensor(
                    out=scr[:], in0=ir[:], scalar=1.0, in1=renv2[:],
                    op0=OP.mult, op1=OP.mult, accum_out=cols[:, 6:7])
                # sc = min(sqrt((ni+1e-8)/(nr+1e-8)), 10)
                nc.vector.tensor_scalar(out=cols[:, 7:8], in0=cols[:, 6:7],
                                        scalar1=1e-8, scalar2=None, op0=OP.add)
                nc.vector.reciprocal(out=cols[:, 8:9], in_=cols[:, 7:8])
                nc.vector.tensor_scalar(out=cols[:, 9:10], in0=cols[:, 2:3],
                                        scalar1=1e-8, scalar2=None, op0=OP.add)
                nc.vector.tensor_tensor(out=cols[:, 10:11], in0=cols[:, 8:9],
                                        in1=cols[:, 9:10], op=OP.mult)
                nc.scalar.activation(out=cols[:, 11:12], in_=cols[:, 10:11], func=AF.Sqrt)
                nc.vector.tensor_scalar(out=cols[:, 12:13], in0=cols[:, 11:12],
                                        scalar1=10.0, scalar2=None, op0=OP.min)
                fac = tmp.tile([128, D], f32, tag="u")
                nc.vector.tensor_scalar(out=fac[:], in0=renv[:],
                                        scalar1=cols[:, 12:13], scalar2=None,
                                        op0=OP.mult)
                # rot_r = (pr*cos_p - pi*sin_p)*fac ; rot_i = (pr*sin_p + pi*cos_p)*fac
                t1 = tmp.tile([128, D], f32, tag="ta")
                t2 = tmp.tile([128, D], f32, tag="tb")
                nc.vector.tensor_tensor(out=t1[:], in0=prt[:], in1=cp[:], op=OP.mult)
                nc.gpsimd.tensor_tensor(out=t2[:], in0=pit[:], in1=sp[:], op=OP.mult)
                Rt = tmp.tile([128, D], f32, tag="Rt")
                nc.vector.tensor_tensor(out=Rt[:], in0=t1[:], in1=t2[:], op=OP.subtract)
                t3 = tmp.tile([128, D], f32, tag="ta")
                t4 = tmp.tile([128, D], f32, tag="tb")
                nc.gpsimd.tensor_tensor(out=t3[:], in0=prt[:], in1=sp[:], op=OP.mult)
                nc.vector.tensor_tensor(out=t4[:], in0=pit[:], in1=cp[:], op=OP.mult)
                I2t = tmp.tile([128, D], f32, tag="cp")
                nc.vector.tensor_tensor(out=I2t[:], in0=t3[:], in1=t4[:], op=OP.add)
                rr = tmp.tile([128, D], f32, tag="sp")
                nc.vector.tensor_tensor(out=rr[:], in0=Rt[:], in1=fac[:], op=OP.mult)
                ri = tmp.tile([128, D], f32, tag="q2")
                nc.gpsimd.tensor_tensor(out=ri[:], in0=I2t[:], in1=fac[:], op=OP.mult)

                # ======== stage A2: transpose + forward DFT ========
                rrT = trTp.tile([128, D], f32, tag="rrT")
                riT = trTp.tile([128, D], f32, tag="riT")
                for k in range(KCH):
                    pt = pst.tile([128, 128], f32, tag="pt")
                    nc.tensor.transpose(pt[:], rr[:, k * 128:(k + 1) * 128], ident[:])
                    nc.scalar.copy(rrT[:, k * 128:(k + 1) * 128], pt[:])
                    pt2 = pst.tile([128, 128], f32, tag="pt")
                    nc.tensor.transpose(pt2[:], ri[:, k * 128:(k + 1) * 128], ident[:])
                    nc.vector.tensor_copy(riT[:, k * 128:(k + 1) * 128], pt2[:])

                br = bhp.tile([128, D], f32, tag="br")
                bi = bhp.tile([128, D], f32, tag="bi")
                pbr = psf.tile([128, D], f32, tag="pbr")
                for k in range(KCH):
                    nc.tensor.matmul(pbr[:], rrT[:, k * 128:(k + 1) * 128],
                                     mats["PT"][:, k * D:(k + 1) * D],
                                     start=(k == 0), stop=False)
                for k in range(KCH):
                    nc.tensor.matmul(pbr[:], riT[:, k * 128:(k + 1) * 128],
                                     mats["QTn"][:, k * D:(k + 1) * D],
                                     start=False, stop=(k == KCH - 1))
                nc.scalar.copy(br[:], pbr[:])
                pbi = psf.tile([128, D], f32, tag="pbi")
                for k in range(KCH):
                    nc.tensor.matmul(pbi[:], rrT[:, k * 128:(k + 1) * 128],
                                     mats["QT"][:, k * D:(k + 1) * D],
                                     start=(k == 0), stop=False)
                for k in range(KCH):
                    nc.tensor.matmul(pbi[:], riT[:, k * 128:(k + 1) * 128],
                                     mats["PT"][:, k * D:(k + 1) * D],
                                     start=False, stop=(k == KCH - 1))
                nc.vector.tensor_copy(bi[:], pbi[:])

                # ======== stage B: CG in frequency space ========
                # planes: p_r, p_i, xh_r, xh_i, Apr, Api, w1, w2; r lives in br/bi
                p_r = cgp.tile([128, D], f32, tag="p_r")
                p_i = cgp.tile([128, D], f32, tag="p_i")
                xhr = cgp.tile([128, D], f32, tag="xhr")
                xhi = cgp.tile([128, D], f32, tag="xhi")
                Apr = cgp.tile([128, D], f32, tag="Apr")
                Api = cgp.tile([128, D], f32, tag="Api")
                w1 = cgp.tile([128, D], f32, tag="w1")
                w2 = cgp.tile([128, D], f32, tag="w2")
                sc = scp.tile([128, 288], f32, tag="sc")

                # engine rules: Pool = tensor_tensor only; vector = reductions
                # + small [128,1] algebra; scalar(ACT) = per-system Copy-scale
                # broadcasts (tableless).
                nc.vector.memset(xhr[:], 0.0)
                nc.vector.memset(xhi[:], 0.0)
                # rz0 = (inv_s2/D) * sum(|bhat|^2) ; p0 = m * bhat
                nc.vector.scalar_tensor_tensor(
                    out=w1[:], in0=br[:], scalar=1.0, in1=br[:],
                    op0=OP.mult, op1=OP.mult, accum_out=sc[:, 0:1])
                nc.vector.scalar_tensor_tensor(
                    out=w2[:], in0=bi[:], scalar=1.0, in1=bi[:],
                    op0=OP.mult, op1=OP.mult, accum_out=sc[:, 1:2])
                nc.gpsimd.tensor_tensor(out=sc[:, 2:3], in0=sc[:, 0:1],
                                        in1=sc[:, 1:2], op=OP.add)
                nc.vector.tensor_scalar(out=sc[:, 3:4], in0=sc[:, 2:3],
                                        scalar1=c_isd, scalar2=None, op0=OP.mult)
                nc.scalar.activation(out=p_r[:], in_=br[:], func=AF.Copy, scale=c_is2)
                nc.scalar.activation(out=Api[:], in_=bi[:], func=AF.Copy, scale=c_nds)
                nc.gpsimd.tensor_tensor(out=p_r[:], in0=p_r[:], in1=Api[:], op=OP.add)
                nc.scalar.activation(out=p_i[:], in_=bi[:], func=AF.Copy, scale=c_is2)
                nc.scalar.activation(out=Apr[:], in_=br[:], func=AF.Copy, scale=c_pds)
                nc.gpsimd.tensor_tensor(out=p_i[:], in0=p_i[:], in1=Apr[:], op=OP.add)

                rz = sc[:, 3:4]
                for it in range(NIT):
                    cb = 4 + it * 14
                    def col(j, cb=cb):
                        return sc[:, cb + j:cb + j + 1]
                    # Ap = (1 + i*kl) p
                    nc.gpsimd.tensor_tensor(out=w1[:], in0=KL[:], in1=p_i[:], op=OP.mult)
                    nc.gpsimd.tensor_tensor(out=Apr[:], in0=p_r[:], in1=w1[:], op=OP.subtract)
                    nc.gpsimd.tensor_tensor(out=w2[:], in0=KL[:], in1=p_r[:], op=OP.mult)
                    nc.gpsimd.tensor_tensor(out=Api[:], in0=p_i[:], in1=w2[:], op=OP.add)
                    # pAp = (1/D)*(sum(p_r*Apr) + sum(p_i*Api)) ; a = rz/(pAp+eps)
                    nc.vector.scalar_tensor_tensor(
                        out=w1[:], in0=p_r[:], scalar=1.0 / D, in1=Apr[:],
                        op0=OP.mult, op1=OP.mult, accum_out=col(0))
                    nc.vector.scalar_tensor_tensor(
                        out=w2[:], in0=p_i[:], scalar=1.0 / D, in1=Api[:],
                        op0=OP.mult, op1=OP.mult, accum_out=col(1))
                    nc.vector.scalar_tensor_tensor(
                        out=col(3), in0=col(0), scalar=1e-30, in1=col(1),
                        op0=OP.add, op1=OP.add)
                    nc.vector.reciprocal(out=col(4), in_=col(3))
                    nc.gpsimd.tensor_tensor(out=col(5), in0=rz, in1=col(4), op=OP.mult)  # a
                    nc.scalar.activation(out=col(6), in_=col(5), func=AF.Copy,
                                         scale=-1.0)                                     # -a
                    # x += a*p ; r -= a*Ap  (scalar engine broadcasts, Pool adds)
                    nc.scalar.activation(out=w1[:], in_=p_r[:], func=AF.Copy, scale=col(5))
                    nc.gpsimd.tensor_tensor(out=xhr[:], in0=xhr[:], in1=w1[:], op=OP.add)
                    nc.scalar.activation(out=w2[:], in_=p_i[:], func=AF.Copy, scale=col(5))
                    nc.gpsimd.tensor_tensor(out=xhi[:], in0=xhi[:], in1=w2[:], op=OP.add)
                    nc.scalar.activation(out=Apr[:], in_=Apr[:], func=AF.Copy, scale=col(6))
                    nc.gpsimd.tensor_tensor(out=br[:], in0=br[:], in1=Apr[:], op=OP.add)
                    nc.scalar.activation(out=Api[:], in_=Api[:], func=AF.Copy, scale=col(6))
                    nc.gpsimd.tensor_tensor(out=bi[:], in0=bi[:], in1=Api[:], op=OP.add)
                    # rz_new = (inv_s2/D) * ||r||^2 ; beta = rz_new/(rz+eps)
                    nc.vector.scalar_tensor_tensor(
                        out=w1[:], in0=br[:], scalar=1.0, in1=br[:],
                        op0=OP.mult, op1=OP.mult, accum_out=col(7))
                    nc.vector.scalar_tensor_tensor(
                        out=w2[:], in0=bi[:], scalar=1.0, in1=bi[:],
                        op0=OP.mult, op1=OP.mult, accum_out=col(8))
                    nc.gpsimd.tensor_tensor(out=col(9), in0=col(7), in1=col(8), op=OP.add)
                    nc.vector.tensor_scalar(out=col(10), in0=col(9),
                                            scalar1=c_isd, scalar2=None, op0=OP.mult)
                    nc.vector.tensor_scalar(out=col(11), in0=rz,
                                            scalar1=1e-30, scalar2=None, op0=OP.add)
                    nc.vector.reciprocal(out=col(12), in_=col(11))
                    nc.gpsimd.tensor_tensor(out=col(13), in0=col(10), in1=col(12),
                                            op=OP.mult)  # beta
                    # p = m*r + beta*p
                    nc.scalar.activation(out=Apr[:], in_=br[:], func=AF.Copy, scale=c_is2)
                    nc.scalar.activation(out=Api[:], in_=bi[:], func=AF.Copy, scale=c_nds)
                    nc.gpsimd.tensor_tensor(out=Apr[:], in0=Apr[:], in1=Api[:], op=OP.add)
                    nc.scalar.activation(out=w1[:], in_=p_r[:], func=AF.Copy, scale=col(13))
                    nc.gpsimd.tensor_tensor(out=p_r[:], in0=Apr[:], in1=w1[:], op=OP.add)
                    nc.scalar.activation(out=Apr[:], in_=bi[:], func=AF.Copy, scale=c_is2)
                    nc.scalar.activation(out=Api[:], in_=br[:], func=AF.Copy, scale=c_pds)
                    nc.gpsimd.tensor_tensor(out=Apr[:], in0=Apr[:], in1=Api[:], op=OP.add)
                    nc.scalar.activation(out=w2[:], in_=p_i[:], func=AF.Copy, scale=col(13))
                    nc.gpsimd.tensor_tensor(out=p_i[:], in0=Apr[:], in1=w2[:], op=OP.add)
                    rz = col(10)

                # ======== stage C: inverse DFT + interleave + store ========
                xrT = xtp.tile([128, D], f32, tag="xrT")
                xiT = xtp.tile([128, D], f32, tag="xiT")
                for k in range(KCH):
                    pt = pst.tile([128, 128], f32, tag="pt")
                    nc.tensor.transpose(pt[:], xhr[:, k * 128:(k + 1) * 128], ident[:])
                    nc.scalar.copy(xrT[:, k * 128:(k + 1) * 128], pt[:])
                    pt2 = pst.tile([128, 128], f32, tag="pt")
                    nc.tensor.transpose(pt2[:], xhi[:, k * 128:(k + 1) * 128], ident[:])
                    nc.vector.tensor_copy(xiT[:, k * 128:(k + 1) * 128], pt2[:])

                pxr = psx.tile([128, D], f32, tag="pxr")
                for k in range(KCH):
                    nc.tensor.matmul(pxr[:], xrT[:, k * 128:(k + 1) * 128],
                                     mats["GR"][:, k * D:(k + 1) * D],
                                     start=(k == 0), stop=False)
                for k in range(KCH):
                    nc.tensor.matmul(pxr[:], xiT[:, k * 128:(k + 1) * 128],
                                     mats["GIn"][:, k * D:(k + 1) * D],
                                     start=False, stop=(k == KCH - 1))
                pxi = psx.tile([128, D], f32, tag="pxi")
                for k in range(KCH):
                    nc.tensor.matmul(pxi[:], xrT[:, k * 128:(k + 1) * 128],
                                     mats["GI"][:, k * D:(k + 1) * D],
                                     start=(k == 0), stop=False)
                for k in range(KCH):
                    nc.tensor.matmul(pxi[:], xiT[:, k * 128:(k + 1) * 128],
                                     mats["GR"][:, k * D:(k + 1) * D],
                                     start=False, stop=(k == KCH - 1))
                ot = outp.tile([128, 2 * D], f16, tag="ot")
                ov = ot[:].rearrange("p (d t) -> p d t", t=2)
                nc.scalar.copy(ov[:, :, 0], pxr[:])
                nc.vector.tensor_copy(ov[:, :, 1], pxi[:])
                nc.sync.dma_start(x_d[rows, :], ot[:])
    nc.compile()
    return nc


_cache = {}


def _make_exec(nc, replicated=()):
    """Sharded jit runner for a Bass module; global arrays in/out.

    Mirrors bass2jax.run_bass_via_pjrt's multi-core path but keeps the jitted
    callable so constants can stay device-resident between calls, and takes
    pre-sharded donated zero output buffers (cheap, created on-device)."""
    import jax
    from jax.sharding import Mesh, PartitionSpec, NamedSharding
    from concourse import bass2jax, mybir as _mb

    bass2jax.install_neuronx_cc_hook()
    partition_name = (nc.partition_id_tensor.name
                      if nc.partition_id_tensor else None)
    in_names, out_names, out_avals, out_shapes = [], [], [], []
    for alloc in nc.m.functions[0].allocations:
        if not isinstance(alloc, _mb.MemoryLocationSet):
            continue
        name = alloc.memorylocations[0].name
        if alloc.kind == "ExternalInput":
            if name != partition_name:
                in_names.append(name)
        elif alloc.kind == "ExternalOutput":
            out_names.append(name)
            shape = tuple(alloc.tensor_shape)
            dtype = _mb.dt.np(alloc.dtype)
            out_avals.append(jax.core.ShapedArray(shape, dtype))
            out_shapes.append(((NCORES * shape[0],) + shape[1:], dtype))
    n_params = len(in_names)
    all_in = list(in_names) + list(out_names)
    if partition_name is not None:
        all_in.append(partition_name)

    def _body(*args):
        operands = list(args)
        if partition_name is not None:
            operands.append(bass2jax.partition_id_tensor())
        return tuple(bass2jax._bass_exec_p.bind(
            *operands,
            out_avals=tuple(out_avals),
            in_names=tuple(all_in),
            out_names=tuple(out_names),
            lowering_input_output_aliases=(),
            sim_require_finite=True,
            sim_require_nnan=True,
            nc=nc,
        ))

    devices = jax.devices()[:NCORES]
    mesh = Mesh(np.asarray(devices), ("core",))
    n_outs = len(out_names)
    from jax.experimental.shard_map import shard_map
    in_specs = tuple(
        PartitionSpec() if nm in replicated else PartitionSpec("core")
        for nm in in_names
    ) + (PartitionSpec("core"),) * n_outs
    sharded = jax.jit(
        shard_map(_body, mesh=mesh,
                  in_specs=in_specs,
                  out_specs=(PartitionSpec("core"),) * n_outs,
                  check_rep=False),
        keep_unused=True,
    )

    # The output operands are an ABI placeholder: the NEFF binds outputs to
    # the custom-call RESULTS (fresh buffers), and this kernel writes every
    # output element, so the placeholder content never matters.  Create it
    # once on-device and reuse (no donation), avoiding a per-call dispatch.
    import jax.numpy as jnp
    zs = [
        jax.jit(lambda shp=shp, dt=dt: jnp.zeros(shp, dt),
                out_shardings=NamedSharding(mesh, PartitionSpec("core")))()
        for shp, dt in out_shapes
    ]
    for z in zs:
        z.block_until_ready()

    def run(feed):  # feed: dict name -> global array (np or jax)
        args = [feed[n] for n in in_names]
        return sharded(*args, *zs)

    return run, out_names, mesh


def kernel(psi_r, psi_i, alpha, edge_weights):
    psi_r = np.asarray(psi_r, np.float32).reshape(N, D)
    psi_i = np.asarray(psi_i, np.float32).reshape(N, D)
    alpha = np.asarray(alpha, np.float64)
    edge_weights = np.asarray(edge_weights, np.float64)
    try:
        return _kernel_fast(psi_r, psi_i, alpha, edge_weights)
    except Exception:
        return _kernel_safe(psi_r, psi_i, alpha, edge_weights)


def _get_consts_dev(alpha, edge_weights, mesh):
    """Device-resident replicated constant tensors, cached by content."""
    import jax
    from jax.sharding import NamedSharding, PartitionSpec
    key = (edge_weights.tobytes(), alpha.tobytes())
    ent = _cache.get("consts")
    if ent is not None and ent[0] == key:
        return ent[1]
    c = _host_const(edge_weights, alpha)
    rep = NamedSharding(mesh, PartitionSpec())
    dev = {k: jax.device_put(c[k], rep) for k in CONST_NAMES}
    _cache["consts"] = (key, dev)
    return dev


def _pool():
    import concurrent.futures as cf
    if "pool" not in _cache:
        _cache["pool"] = cf.ThreadPoolExecutor(8)
    return _cache["pool"]


def _kernel_fast(psi_r, psi_i, alpha, edge_weights):
    import jax
    from jax.sharding import NamedSharding, PartitionSpec
    if "fused" not in _cache:
        _cache["fused"] = _build_fused(ntile=NTILE)
    if "ex" not in _cache:
        _cache["ex"] = _make_exec(_cache["fused"], replicated=CONST_NAMES)
    run, out_names, mesh = _cache["ex"]
    consts = _get_consts_dev(alpha, edge_weights, mesh)
    pool = _pool()
    sh = NamedSharding(mesh, PartitionSpec("core"))

    # parallel host casts, then async uploads; dispatch immediately so the
    # dispatch RPC overlaps the transfers
    fa = pool.submit(lambda: psi_r.astype(np.float16))
    fb = pool.submit(lambda: psi_i.astype(np.float16))
    da = jax.device_put(fa.result(), sh)
    db = jax.device_put(fb.result(), sh)

    feed = dict(consts)
    feed["pr16"] = da
    feed["pi16"] = db
    h = dict(zip(out_names, run(feed)))["xout"]

    # parallel per-shard fetch (pure f16 pull, no GIL-bound cast in the
    # fetch threads), then a separate parallel upcast pass
    shards = h.addressable_shards
    for s in shards:
        s.data.copy_to_host_async()
    parts = list(pool.map(
        lambda s: (s.index[0].start or 0, np.asarray(s.data)), shards))
    final = np.empty((N, 2 * D), np.float32)

    def upcast(p):
        i0, a = p
        final[i0:i0 + a.shape[0]] = a
    list(pool.map(upcast, parts))
    return final.reshape(B, S, D, 2)


def _kernel_safe(psi_r, psi_i, alpha, edge_weights):
    if "fused" not in _cache:
        _cache["fused"] = _build_fused(ntile=NTILE)
    knl = _cache["fused"]
    c = _host_const(edge_weights, alpha)
    pr16 = psi_r.astype(np.float16)
    pi16 = psi_i.astype(np.float16)
    core_ids = list(range(NCORES))
    ins = []
    for ci in core_ids:
        rows = slice(ci * NSYS, (ci + 1) * NSYS)
        d = {k: c[k] for k in CONST_NAMES}
        d["pr16"] = pr16[rows]
        d["pi16"] = pi16[rows]
        ins.append(d)
    res = run_bass_kernel_spmd(knl, ins, core_ids)
    x16 = np.concatenate([res.results[ci]["xout"] for ci in core_ids], axis=0)
    return x16.astype(np.float32).reshape(B, S, D, 2)


# revision 16
# speedup vs baseline: 1.3295x; 1.3295x over previous
"""Cayley soliton propagator — fused single-pass Trainium2 Bass kernel.

Math: the reference runs 20 PCG iterations on (I + i*k*H) x = (I - i*k*H)
rot(psi) per (batch,token) system, where H is a fixed circulant stencil along
D.  H diagonalizes in the DFT basis with eigenvalues lam_f, so in frequency
space A = 1 + i*k*lam_f is DIAGONAL and every CG iterate is elementwise over
frequencies, with only per-system reductions (dot products over f).  The whole
pipeline therefore fuses into ONE device kernel:

  elementwise phase rotation -> forward modified DFT (PE matmul, the
  (1 - i*k*lam) rhs factor folded into the matrix) -> 20 CG iterations in
  frequency space (vector/pool/scalar engines, diagonal ops + per-system
  accumulations) -> inverse DFT (PE) -> interleaved [.., D, 2] output.

No host round-trip, no collectives.  I/O travels as fp16 (the axon tunnel is
~70 MB/s, so halving bytes halves wall time); all device compute is f32.

Sharding: data-parallel over the flattened system axis N=B*S across 8 cores.
"""

import sys

for _p in ("/opt/trn_rl_repo",):
    if _p not in sys.path:
        sys.path.insert(0, _p)

import numpy as np
import concourse.bass as bass
import concourse.tile as tile
from concourse import bacc, mybir
from concourse.bass_utils import run_bass_kernel_spmd
from concourse.masks import make_identity

f32 = mybir.dt.float32
f16 = mybir.dt.float16
OP = mybir.AluOpType
AF = mybir.ActivationFunctionType

# ---- problem constants (hardcoded per contract) ----
B, S, D = 4, 4096, 512
N = B * S                       # 16384 systems
NCORES = 8
NSYS = N // NCORES              # 2048 systems per core
NTILE = NSYS // 128             # 16 sys-tiles of 128 per core
DT = 0.1
KAP = DT / 2.0                  # 0.05
NIT = 20
NUM_SCALES, BASE_SPARSITY = 3, 5
OFFSETS = [(2 ** s) * j for s in range(NUM_SCALES) for j in range(1, BASE_SPARSITY + 1)]
KCH = 4                         # 512/128 chunks


def _host_const(edge_weights, alpha):
    """Constant tensors for the kernel (computed in fp64, shipped as f32)."""
    w = edge_weights.reshape(-1).astype(np.float64)
    f = np.arange(D)
    deg = 2.0 * w.sum()
    lam = deg - sum(w[k] * 2.0 * np.cos(2 * np.pi * OFFSETS[k] * f / D)
                    for k in range(len(w)))
    dk = KAP * deg
    inv_s2 = 1.0 / (1.0 + dk * dk)

    dmat = np.outer(f, f)
    F = np.exp(-2j * np.pi * dmat / D)              # F[f, d], symmetric
    Fp = (1.0 - 1j * KAP * lam)[:, None] * F        # rhs factor folded in
    PT = np.ascontiguousarray(Fp.real.T)            # [d, f] rhs for fwd matmul
    QT = np.ascontiguousarray(Fp.imag.T)
    QTn = np.ascontiguousarray(-Fp.imag.T)
    G = np.exp(2j * np.pi * dmat / D) / D           # inverse DFT [f, d], symmetric
    GR = np.ascontiguousarray(G.real)
    GI = np.ascontiguousarray(G.imag)
    GIn = np.ascontiguousarray(-G.imag)
    kl = (KAP * lam).reshape(1, D)
    aabs = np.abs(alpha.astype(np.float64)).reshape(1, D)
    cc = np.zeros((1, D))
    cc[0, :4] = [inv_s2, -dk * inv_s2, dk * inv_s2, inv_s2 / D]
    c = dict(PT=PT, QT=QT, QTn=QTn, GR=GR, GI=GI, GIn=GIn,
             kl=kl, aabs=aabs, cc=cc)
    return {k: np.ascontiguousarray(v.astype(np.float32)) for k, v in c.items()}


CONST_NAMES = ("PT", "QT", "QTn", "GR", "GI", "GIn", "kl", "aabs", "cc")


# --------------------------------------------------------------- fused kernel
def _build_fused(ntile=NTILE):
    nsys = ntile * 128
    nc = bacc.Bacc()
    pr_d = nc.declare_dram_parameter("pr16", [nsys, D], f16, isOutput=False)
    pi_d = nc.declare_dram_parameter("pi16", [nsys, D], f16, isOutput=False)
    mat_d = {m: nc.declare_dram_parameter(m, [D, D], f32, isOutput=False)
             for m in ("PT", "QT", "QTn", "GR", "GI", "GIn")}
    kl_d = nc.declare_dram_parameter("kl", [1, D], f32, isOutput=False)
    aa_d = nc.declare_dram_parameter("aabs", [1, D], f32, isOutput=False)
    cc_d = nc.declare_dram_parameter("cc", [1, D], f32, isOutput=False)
    x_d = nc.declare_dram_parameter("xout", [nsys, 2 * D], f16, isOutput=True)

    with tile.TileContext(nc) as tc:
        with tc.tile_pool(name="singles", bufs=1) as singles, \
             tc.tile_pool(name="io", bufs=3) as io, \
             tc.tile_pool(name="tmp", bufs=2) as tmp, \
             tc.tile_pool(name="cols", bufs=2) as colsp, \
             tc.tile_pool(name="trT", bufs=2) as trTp, \
             tc.tile_pool(name="bh", bufs=2) as bhp, \
             tc.tile_pool(name="cg", bufs=2) as cgp, \
             tc.tile_pool(name="sc", bufs=2) as scp, \
             tc.tile_pool(name="xt", bufs=2) as xtp, \
             tc.tile_pool(name="outp", bufs=3) as outp, \
             tc.tile_pool(name="pst", bufs=2, space="PSUM") as pst, \
             tc.tile_pool(name="psf", bufs=1, space="PSUM") as psf, \
             tc.tile_pool(name="psx", bufs=1, space="PSUM") as psx:

            # ---- constants into SBUF ----
            mats = {}
            for m in ("PT", "QT", "QTn", "GR", "GI", "GIn"):
                t = singles.tile([128, KCH * D], f32, name=m)
                for k in range(KCH):
                    nc.sync.dma_start(t[:, k * D:(k + 1) * D],
                                      mat_d[m][k * 128:(k + 1) * 128, :])
                mats[m] = t
            KL = singles.tile([128, D], f32, name="KL")
            nc.gpsimd.dma_start(out=KL[:], in_=kl_d[:].to_broadcast([128, D]))
            aab = singles.tile([128, D], f32, name="aab")
            nc.gpsimd.dma_start(out=aab[:], in_=aa_d[:].to_broadcast([128, D]))
            CC = singles.tile([128, D], f32, name="CC")
            nc.gpsimd.dma_start(out=CC[:], in_=cc_d[:].to_broadcast([128, D]))
            ident = singles.tile([128, 128], f32, name="ident")
            make_identity(nc, ident[:])
            nhalfpi = singles.tile([128, 1], f32, name="nhalfpi")
            nc.vector.memset(nhalfpi[:], float(-np.pi / 2))
            c_is2 = CC[:, 0:1]       # inv_s2
            c_nds = CC[:, 1:2]       # -dk*inv_s2
            c_pds = CC[:, 2:3]       # +dk*inv_s2
            c_isd = CC[:, 3:4]       # inv_s2/D

            for t0 in range(ntile):
                rows = slice(t0 * 128, (t0 + 1) * 128)

                # ======== stage A: load + phase rotation ========
                pr16 = io.tile([128, D], f16, tag="pr16")
                pi16 = io.tile([128, D], f16, tag="pi16")
                nc.sync.dma_start(pr16[:], pr_d[rows, :])
                nc.sync.dma_start(pi16[:], pi_d[rows, :])
                prt = tmp.tile([128, D], f32, tag="prt")
                pit = tmp.tile([128, D], f32, tag="pit")
                nc.scalar.copy(prt[:], pr16[:])
                nc.vector.tensor_copy(pit[:], pi16[:])

                cols = colsp.tile([128, 16], f32, tag="cols")
                ta = tmp.tile([128, D], f32, tag="ta")
                tb = tmp.tile([128, D], f32, tag="tb")
                nc.vector.scalar_tensor_tensor(
                    out=ta[:], in0=prt[:], scalar=1.0, in1=prt[:],
                    op0=OP.mult, op1=OP.mult, accum_out=cols[:, 0:1])
                nc.vector.scalar_tensor_tensor(
                    out=tb[:], in0=pit[:], scalar=1.0, in1=pit[:],
                    op0=OP.mult, op1=OP.mult, accum_out=cols[:, 1:2])
                ir = tmp.tile([128, D], f32, tag="ir")
                nc.gpsimd.tensor_tensor(out=ir[:], in0=ta[:], in1=tb[:], op=OP.add)
                # norm_in = c0+c1 ; rm = 1/max(norm_in/D, 1e-6) ; nrm = -rm
                nc.vector.tensor_tensor(out=cols[:, 2:3], in0=cols[:, 0:1],
                                        in1=cols[:, 1:2], op=OP.add)
                nc.vector.tensor_scalar(out=cols[:, 3:4], in0=cols[:, 2:3],
                                        scalar1=1.0 / D, scalar2=1e-6,
                                        op0=OP.mult, op1=OP.max)
                nc.vector.reciprocal(out=cols[:, 4:5], in_=cols[:, 3:4])
                nc.vector.tensor_scalar(out=cols[:, 5:6], in0=cols[:, 4:5],
                                        scalar1=-1.0, scalar2=None, op0=OP.mult)
                # u = exp(-ir*rm); half-angle: cos_p = 1-2*sin^2(pi*u-pi/2) ...
                u = tmp.tile([128, D], f32, tag="u")
                nc.scalar.activation(out=u[:], in_=ir[:], func=AF.Exp,
                                     bias=0.0, scale=cols[:, 5:6])
                shalf = tmp.tile([128, D], f32, tag="ta")
                nc.scalar.activation(out=shalf[:], in_=u[:], func=AF.Sin,
                                     bias=nhalfpi[:], scale=float(np.pi))
                chalf = tmp.tile([128, D], f32, tag="tb")
                nc.scalar.activation(out=chalf[:], in_=u[:], func=AF.Sin,
                                     bias=0.0, scale=float(np.pi))
                q1 = tmp.tile([128, D], f32, tag="u")
                nc.vector.tensor_tensor(out=q1[:], in0=shalf[:], in1=shalf[:], op=OP.mult)
                cp = tmp.tile([128, D], f32, tag="cp")
                nc.vector.tensor_scalar(out=cp[:], in0=q1[:], scalar1=-2.0,
                                        scalar2=1.0, op0=OP.mult, op1=OP.add)
                q2 = tmp.tile([128, D], f32, tag="q2")
                nc.gpsimd.tensor_tensor(out=q2[:], in0=shalf[:], in1=chalf[:], op=OP.mult)
                sp = tmp.tile([128, D], f32, tag="sp")
                nc.vector.tensor_scalar(out=sp[:], in0=q2[:], scalar1=-2.0,
                                        scalar2=None, op0=OP.mult)
                # env = min(1 + aabs*(ir*rm)^2, 10) ; renv = 1/env
                tsq = tmp.tile([128, D], f32, tag="ta")
                nc.scalar.activation(out=tsq[:], in_=ir[:], func=AF.Square,
                                     bias=0.0, scale=cols[:, 4:5])
                env = tmp.tile([128, D], f32, tag="tb")
                nc.vector.scalar_tensor_tensor(
                    out=env[:], in0=tsq[:], scalar=1.0, in1=aab[:],
                    op0=OP.mult, op1=OP.mult)
                nc.vector.tensor_scalar(out=env[:], in0=env[:],
                                        scalar1=1.0, scalar2=10.0,
                                        op0=OP.add, op1=OP.min)
                renv = tmp.tile([128, D], f32, tag="renv")
                nc.vector.reciprocal_approx_fast(out=renv[:], in_=env[:])
                renv2 = tmp.tile([128, D], f32, tag="ta")
                nc.scalar.activation(out=renv2[:], in_=renv[:], func=AF.Square)
                # norm_rot = sum(ir * renv^2) (|rot|^2 == ir pointwise)
                scr = tmp.tile([128, D], f32, tag="tb")
                nc.vector.scalar_tensor_tensor(
                    out=scr[:], in0=ir[:], scalar=1.0, in1=renv2[:],
                    op0=OP.mult, op1=OP.mult, accum_out=cols[:, 6:7])
                # sc = min(sqrt((ni+1e-8)/(nr+1e-8)), 10)
                nc.vector.tensor_scalar(out=cols[:, 7:8], in0=cols[:, 6:7],
                                        scalar1=1e-8, scalar2=None, op0=OP.add)
                nc.vector.reciprocal(out=cols[:, 8:9], in_=cols[:, 7:8])
                nc.vector.tensor_scalar(out=cols[:, 9:10], in0=cols[:, 2:3],
                                        scalar1=1e-8, scalar2=None, op0=OP.add)
                nc.vector.tensor_tensor(out=cols[:, 10:11], in0=cols[:, 8:9],
                                        in1=cols[:, 9:10], op=OP.mult)
                nc.scalar.activation(out=cols[:, 11:12], in_=cols[:, 10:11], func=AF.Sqrt)
                nc.vector.tensor_scalar(out=cols[:, 12:13], in0=cols[:, 11:12],
                                        scalar1=10.0, scalar2=None, op0=OP.min)
                fac = tmp.tile([128, D], f32, tag="u")
                nc.vector.tensor_scalar(out=fac[:], in0=renv[:],
                                        scalar1=cols[:, 12:13], scalar2=None,
                                        op0=OP.mult)
                # rot_r = (pr*cos_p - pi*sin_p)*fac ; rot_i = (pr*sin_p + pi*cos_p)*fac
                t1 = tmp.tile([128, D], f32, tag="ta")
                t2 = tmp.tile([128, D], f32, tag="tb")
                nc.vector.tensor_tensor(out=t1[:], in0=prt[:], in1=cp[:], op=OP.mult)
                nc.gpsimd.tensor_tensor(out=t2[:], in0=pit[:], in1=sp[:], op=OP.mult)
                Rt = tmp.tile([128, D], f32, tag="Rt")
                nc.vector.tensor_tensor(out=Rt[:], in0=t1[:], in1=t2[:], op=OP.subtract)
                t3 = tmp.tile([128, D], f32, tag="ta")
                t4 = tmp.tile([128, D], f32, tag="tb")
                nc.gpsimd.tensor_tensor(out=t3[:], in0=prt[:], in1=sp[:], op=OP.mult)
                nc.vector.tensor_tensor(out=t4[:], in0=pit[:], in1=cp[:], op=OP.mult)
                I2t = tmp.tile([128, D], f32, tag="cp")
                nc.vector.tensor_tensor(out=I2t[:], in0=t3[:], in1=t4[:], op=OP.add)
                rr = tmp.tile([128, D], f32, tag="sp")
                nc.vector.tensor_tensor(out=rr[:], in0=Rt[:], in1=fac[:], op=OP.mult)
                ri = tmp.tile([128, D], f32, tag="q2")
                nc.gpsimd.tensor_tensor(out=ri[:], in0=I2t[:], in1=fac[:], op=OP.mult)

                # ======== stage A2: transpose + forward DFT ========
                rrT = trTp.tile([128, D], f32, tag="rrT")
                riT = trTp.tile([128, D], f32, tag="riT")
                for k in range(KCH):
                    pt = pst.tile([128, 128], f32, tag="pt")
                    nc.tensor.transpose(pt[:], rr[:, k * 128:(k + 1) * 128], ident[:])
                    nc.scalar.copy(rrT[:, k * 128:(k + 1) * 128], pt[:])
                    pt2 = pst.tile([128, 128], f32, tag="pt")
                    nc.tensor.transpose(pt2[:], ri[:, k * 128:(k + 1) * 128], ident[:])
                    nc.vector.tensor_copy(riT[:, k * 128:(k + 1) * 128], pt2[:])

                br = bhp.tile([128, D], f32, tag="br")
                bi = bhp.tile([128, D], f32, tag="bi")
                pbr = psf.tile([128, D], f32, tag="pbr")
                for k in range(KCH):
                    nc.tensor.matmul(pbr[:], rrT[:, k * 128:(k + 1) * 128],
                                     mats["PT"][:, k * D:(k + 1) * D],
                                     start=(k == 0), stop=False)
                for k in range(KCH):
                    nc.tensor.matmul(pbr[:], riT[:, k * 128:(k + 1) * 128],
                                     mats["QTn"][:, k * D:(k + 1) * D],
                                     start=False, stop=(k == KCH - 1))
                nc.scalar.copy(br[:], pbr[:])
                pbi = psf.tile([128, D], f32, tag="pbi")
                for k in range(KCH):
                    nc.tensor.matmul(pbi[:], rrT[:, k * 128:(k + 1) * 128],
                                     mats["QT"][:, k * D:(k + 1) * D],
                                     start=(k == 0), stop=False)
                for k in range(KCH):
                    nc.tensor.matmul(pbi[:], riT[:, k * 128:(k + 1) * 128],
                                     mats["PT"][:, k * D:(k + 1) * D],
                                     start=False, stop=(k == KCH - 1))
                nc.vector.tensor_copy(bi[:], pbi[:])

                # ======== stage B: CG in frequency space ========
                # planes: p_r, p_i, xh_r, xh_i, Apr, Api, w1, w2; r lives in br/bi
                p_r = cgp.tile([128, D], f32, tag="p_r")
                p_i = cgp.tile([128, D], f32, tag="p_i")
                xhr = cgp.tile([128, D], f32, tag="xhr")
                xhi = cgp.tile([128, D], f32, tag="xhi")
                Apr = cgp.tile([128, D], f32, tag="Apr")
                Api = cgp.tile([128, D], f32, tag="Api")
                w1 = cgp.tile([128, D], f32, tag="w1")
                w2 = cgp.tile([128, D], f32, tag="w2")
                sc = scp.tile([128, 288], f32, tag="sc")

                # engine rules: Pool = tensor_tensor only; vector = reductions
                # + small [128,1] algebra; scalar(ACT) = per-system Copy-scale
                # broadcasts (tableless).
                nc.vector.memset(xhr[:], 0.0)
                nc.vector.memset(xhi[:], 0.0)
                # rz0 = (inv_s2/D) * sum(|bhat|^2) ; p0 = m * bhat
                nc.vector.scalar_tensor_tensor(
                    out=w1[:], in0=br[:], scalar=1.0, in1=br[:],
                    op0=OP.mult, op1=OP.mult, accum_out=sc[:, 0:1])
                nc.vector.scalar_tensor_tensor(
                    out=w2[:], in0=bi[:], scalar=1.0, in1=bi[:],
                    op0=OP.mult, op1=OP.mult, accum_out=sc[:, 1:2])
                nc.gpsimd.tensor_tensor(out=sc[:, 2:3], in0=sc[:, 0:1],
                                        in1=sc[:, 1:2], op=OP.add)
                nc.vector.tensor_scalar(out=sc[:, 3:4], in0=sc[:, 2:3],
                                        scalar1=c_isd, scalar2=None, op0=OP.mult)
                nc.scalar.activation(out=p_r[:], in_=br[:], func=AF.Copy, scale=c_is2)
                nc.scalar.activation(out=Api[:], in_=bi[:], func=AF.Copy, scale=c_nds)
                nc.gpsimd.tensor_tensor(out=p_r[:], in0=p_r[:], in1=Api[:], op=OP.add)
                nc.scalar.activation(out=p_i[:], in_=bi[:], func=AF.Copy, scale=c_is2)
                nc.scalar.activation(out=Apr[:], in_=br[:], func=AF.Copy, scale=c_pds)
                nc.gpsimd.tensor_tensor(out=p_i[:], in0=p_i[:], in1=Apr[:], op=OP.add)

                rz = sc[:, 3:4]
                for it in range(NIT):
                    cb = 4 + it * 14
                    def col(j, cb=cb):
                        return sc[:, cb + j:cb + j + 1]
                    # Ap = (1 + i*kl) p
                    nc.gpsimd.tensor_tensor(out=w1[:], in0=KL[:], in1=p_i[:], op=OP.mult)
                    nc.gpsimd.tensor_tensor(out=Apr[:], in0=p_r[:], in1=w1[:], op=OP.subtract)
                    nc.gpsimd.tensor_tensor(out=w2[:], in0=KL[:], in1=p_r[:], op=OP.mult)
                    nc.gpsimd.tensor_tensor(out=Api[:], in0=p_i[:], in1=w2[:], op=OP.add)
                    # pAp = (1/D)*(sum(p_r*Apr) + sum(p_i*Api)) ; a = rz/(pAp+eps)
                    nc.vector.scalar_tensor_tensor(
                        out=w1[:], in0=p_r[:], scalar=1.0 / D, in1=Apr[:],
                        op0=OP.mult, op1=OP.mult, accum_out=col(0))
                    nc.vector.scalar_tensor_tensor(
                        out=w2[:], in0=p_i[:], scalar=1.0 / D, in1=Api[:],
                        op0=OP.mult, op1=OP.mult, accum_out=col(1))
                    nc.vector.scalar_tensor_tensor(
                        out=col(3), in0=col(0), scalar=1e-30, in1=col(1),
                        op0=OP.add, op1=OP.add)
                    nc.vector.reciprocal(out=col(4), in_=col(3))
                    nc.gpsimd.tensor_tensor(out=col(5), in0=rz, in1=col(4), op=OP.mult)  # a
                    nc.scalar.activation(out=col(6), in_=col(5), func=AF.Copy,
                                         scale=-1.0)                                     # -a
                    # x += a*p ; r -= a*Ap  (scalar engine broadcasts, Pool adds)
                    nc.scalar.activation(out=w1[:], in_=p_r[:], func=AF.Copy, scale=col(5))
                    nc.gpsimd.tensor_tensor(out=xhr[:], in0=xhr[:], in1=w1[:], op=OP.add)
                    nc.scalar.activation(out=w2[:], in_=p_i[:], func=AF.Copy, scale=col(5))
                    nc.gpsimd.tensor_tensor(out=xhi[:], in0=xhi[:], in1=w2[:], op=OP.add)
                    nc.scalar.activation(out=Apr[:], in_=Apr[:], func=AF.Copy, scale=col(6))
                    nc.gpsimd.tensor_tensor(out=br[:], in0=br[:], in1=Apr[:], op=OP.add)
                    nc.scalar.activation(out=Api[:], in_=Api[:], func=AF.Copy, scale=col(6))
                    nc.gpsimd.tensor_tensor(out=bi[:], in0=bi[:], in1=Api[:], op=OP.add)
                    # rz_new = (inv_s2/D) * ||r||^2 ; beta = rz_new/(rz+eps)
                    nc.vector.scalar_tensor_tensor(
                        out=w1[:], in0=br[:], scalar=1.0, in1=br[:],
                        op0=OP.mult, op1=OP.mult, accum_out=col(7))
                    nc.vector.scalar_tensor_tensor(
                        out=w2[:], in0=bi[:], scalar=1.0, in1=bi[:],
                        op0=OP.mult, op1=OP.mult, accum_out=col(8))
                    nc.gpsimd.tensor_tensor(out=col(9), in0=col(7), in1=col(8), op=OP.add)
                    nc.vector.tensor_scalar(out=col(10), in0=col(9),
                                            scalar1=c_isd, scalar2=None, op0=OP.mult)
                    nc.vector.tensor_scalar(out=col(11), in0=rz,
                                            scalar1=1e-30, scalar2=None, op0=OP.add)
                    nc.vector.reciprocal(out=col(12), in_=col(11))
                    nc.gpsimd.tensor_tensor(out=col(13), in0=col(10), in1=col(12),
                                            op=OP.mult)  # beta
                    # p = m*r + beta*p
                    nc.scalar.activation(out=Apr[:], in_=br[:], func=AF.Copy, scale=c_is2)
                    nc.scalar.activation(out=Api[:], in_=bi[:], func=AF.Copy, scale=c_nds)
                    nc.gpsimd.tensor_tensor(out=Apr[:], in0=Apr[:], in1=Api[:], op=OP.add)
                    nc.scalar.activation(out=w1[:], in_=p_r[:], func=AF.Copy, scale=col(13))
                    nc.gpsimd.tensor_tensor(out=p_r[:], in0=Apr[:], in1=w1[:], op=OP.add)
                    nc.scalar.activation(out=Apr[:], in_=bi[:], func=AF.Copy, scale=c_is2)
                    nc.scalar.activation(out=Api[:], in_=br[:], func=AF.Copy, scale=c_pds)
                    nc.gpsimd.tensor_tensor(out=Apr[:], in0=Apr[:], in1=Api[:], op=OP.add)
                    nc.scalar.activation(out=w2[:], in_=p_i[:], func=AF.Copy, scale=col(13))
                    nc.gpsimd.tensor_tensor(out=p_i[:], in0=Apr[:], in1=w2[:], op=OP.add)
                    rz = col(10)

                # ======== stage C: inverse DFT + interleave + store ========
                xrT = xtp.tile([128, D], f32, tag="xrT")
                xiT = xtp.tile([128, D], f32, tag="xiT")
                for k in range(KCH):
                    pt = pst.tile([128, 128], f32, tag="pt")
                    nc.tensor.transpose(pt[:], xhr[:, k * 128:(k + 1) * 128], ident[:])
                    nc.scalar.copy(xrT[:, k * 128:(k + 1) * 128], pt[:])
                    pt2 = pst.tile([128, 128], f32, tag="pt")
                    nc.tensor.transpose(pt2[:], xhi[:, k * 128:(k + 1) * 128], ident[:])
                    nc.vector.tensor_copy(xiT[:, k * 128:(k + 1) * 128], pt2[:])

                pxr = psx.tile([128, D], f32, tag="pxr")
                for k in range(KCH):
                    nc.tensor.matmul(pxr[:], xrT[:, k * 128:(k + 1) * 128],
                                     mats["GR"][:, k * D:(k + 1) * D],
                                     start=(k == 0), stop=False)
                for k in range(KCH):
                    nc.tensor.matmul(pxr[:], xiT[:, k * 128:(k + 1) * 128],
                                     mats["GIn"][:, k * D:(k + 1) * D],
                                     start=False, stop=(k == KCH - 1))
                pxi = psx.tile([128, D], f32, tag="pxi")
                for k in range(KCH):
                    nc.tensor.matmul(pxi[:], xrT[:, k * 128:(k + 1) * 128],
                                     mats["GI"][:, k * D:(k + 1) * D],
                                     start=(k == 0), stop=False)
                for k in range(KCH):
                    nc.tensor.matmul(pxi[:], xiT[:, k * 128:(k + 1) * 128],
                                     mats["GR"][:, k * D:(k + 1) * D],
                                     start=False, stop=(k == KCH - 1))
                ot = outp.tile([128, 2 * D], f16, tag="ot")
                ov = ot[:].rearrange("p (d t) -> p d t", t=2)
                nc.scalar.copy(ov[:, :, 0], pxr[:])
                nc.vector.tensor_copy(ov[:, :, 1], pxi[:])
                nc.sync.dma_start(x_d[rows, :], ot[:])
    nc.compile()
    return nc


_cache = {}


def _make_exec(nc, replicated=()):
    """Sharded jit runner for a Bass module; global arrays in/out.

    Mirrors bass2jax.run_bass_via_pjrt's multi-core path but keeps the jitted
    callable so constants can stay device-resident between calls, and takes
    pre-sharded donated zero output buffers (cheap, created on-device)."""
    import jax
    from jax.sharding import Mesh, PartitionSpec, NamedSharding
    from concourse import bass2jax, mybir as _mb

    bass2jax.install_neuronx_cc_hook()
    partition_name = (nc.partition_id_tensor.name
                      if nc.partition_id_tensor else None)
    in_names, out_names, out_avals, out_shapes = [], [], [], []
    for alloc in nc.m.functions[0].allocations:
        if not isinstance(alloc, _mb.MemoryLocationSet):
            continue
        name = alloc.memorylocations[0].name
        if alloc.kind == "ExternalInput":
            if name != partition_name:
                in_names.append(name)
        elif alloc.kind == "ExternalOutput":
            out_names.append(name)
            shape = tuple(alloc.tensor_shape)
            dtype = _mb.dt.np(alloc.dtype)
            out_avals.append(jax.core.ShapedArray(shape, dtype))
            out_shapes.append(((NCORES * shape[0],) + shape[1:], dtype))
    n_params = len(in_names)
    all_in = list(in_names) + list(out_names)
    if partition_name is not None:
        all_in.append(partition_name)

    def _body(*args):
        operands = list(args)
        if partition_name is not None:
            operands.append(bass2jax.partition_id_tensor())
        return tuple(bass2jax._bass_exec_p.bind(
            *operands,
            out_avals=tuple(out_avals),
            in_names=tuple(all_in),
            out_names=tuple(out_names),
            lowering_input_output_aliases=(),
            sim_require_finite=True,
            sim_require_nnan=True,
            nc=nc,
        ))

    devices = jax.devices()[:NCORES]
    mesh = Mesh(np.asarray(devices), ("core",))
    n_outs = len(out_names)
    from jax.experimental.shard_map import shard_map
    in_specs = tuple(
        PartitionSpec() if nm in replicated else PartitionSpec("core")
        for nm in in_names
    ) + (PartitionSpec("core"),) * n_outs
    sharded = jax.jit(
        shard_map(_body, mesh=mesh,
                  in_specs=in_specs,
                  out_specs=(PartitionSpec("core"),) * n_outs,
                  check_rep=False),
        keep_unused=True,
    )

    # The output operands are an ABI placeholder: the NEFF binds outputs to
    # the custom-call RESULTS (fresh buffers), and this kernel writes every
    # output element, so the placeholder content never matters.  Create it
    # once on-device and reuse (no donation), avoiding a per-call dispatch.
    import jax.numpy as jnp
    zs = [
        jax.jit(lambda shp=shp, dt=dt: jnp.zeros(shp, dt),
                out_shardings=NamedSharding(mesh, PartitionSpec("core")))()
        for shp, dt in out_shapes
    ]
    for z in zs:
        z.block_until_ready()

    def run(feed):  # feed: dict name -> global array (np or jax)
        args = [feed[n] for n in in_names]
        return sharded(*args, *zs)

    return run, out_names, mesh


def kernel(psi_r, psi_i, alpha, edge_weights):
    psi_r = np.asarray(psi_r, np.float32).reshape(N, D)
    psi_i = np.asarray(psi_i, np.float32).reshape(N, D)
    alpha = np.asarray(alpha, np.float64)
    edge_weights = np.asarray(edge_weights, np.float64)
    try:
        return _kernel_fast(psi_r, psi_i, alpha, edge_weights)
    except Exception:
        return _kernel_safe(psi_r, psi_i, alpha, edge_weights)


def _get_consts_dev(alpha, edge_weights, mesh):
    """Device-resident replicated constant tensors, cached by content."""
    import jax
    from jax.sharding import NamedSharding, PartitionSpec
    key = (edge_weights.tobytes(), alpha.tobytes())
    ent = _cache.get("consts")
    if ent is not None and ent[0] == key:
        return ent[1]
    c = _host_const(edge_weights, alpha)
    rep = NamedSharding(mesh, PartitionSpec())
    dev = {k: jax.device_put(c[k], rep) for k in CONST_NAMES}
    _cache["consts"] = (key, dev)
    return dev


def _pool():
    import concurrent.futures as cf
    if "pool" not in _cache:
        _cache["pool"] = cf.ThreadPoolExecutor(8)
    return _cache["pool"]


def _kernel_fast(psi_r, psi_i, alpha, edge_weights):
    import jax
    from jax.sharding import NamedSharding, PartitionSpec
    if "fused" not in _cache:
        _cache["fused"] = _build_fused(ntile=NTILE)
    if "ex" not in _cache:
        _cache["ex"] = _make_exec(_cache["fused"], replicated=CONST_NAMES)
    run, out_names, mesh = _cache["ex"]
    consts = _get_consts_dev(alpha, edge_weights, mesh)
    pool = _pool()
    sh = NamedSharding(mesh, PartitionSpec("core"))

    # parallel host casts, then async uploads; dispatch immediately so the
    # dispatch RPC overlaps the transfers
    fa = pool.submit(lambda: psi_r.astype(np.float16))
    fb = pool.submit(lambda: psi_i.astype(np.float16))
    da = jax.device_put(fa.result(), sh)
    db = jax.device_put(fb.result(), sh)

    feed = dict(consts)
    feed["pr16"] = da
    feed["pi16"] = db
    h = dict(zip(out_names, run(feed)))["xout"]

    # parallel per-shard fetch (pure f16 pull in the pool threads); the main
    # thread upcasts each part as it lands, overlapping cast with fetch
    import queue as _q
    shards = h.addressable_shards
    for s in shards:
        s.data.copy_to_host_async()
    doneq = _q.Queue()

    def fetch(s):
        doneq.put((s.index[0].start or 0, np.asarray(s.data)))
    for s in shards:
        pool.submit(fetch, s)
    final = np.empty((N, 2 * D), np.float32)
    for _ in range(len(shards)):
        i0, a = doneq.get()
        final[i0:i0 + a.shape[0]] = a
    return final.reshape(B, S, D, 2)


def _kernel_safe(psi_r, psi_i, alpha, edge_weights):
    if "fused" not in _cache:
        _cache["fused"] = _build_fused(ntile=NTILE)
    knl = _cache["fused"]
    c = _host_const(edge_weights, alpha)
    pr16 = psi_r.astype(np.float16)
    pi16 = psi_i.astype(np.float16)
    core_ids = list(range(NCORES))
    ins = []
    for ci in core_ids:
        rows = slice(ci * NSYS, (ci + 1) * NSYS)
        d = {k: c[k] for k in CONST_NAMES}
        d["pr16"] = pr16[rows]
        d["pi16"] = pi16[rows]
        ins.append(d)
    res = run_bass_kernel_spmd(knl, ins, core_ids)
    x16 = np.concatenate([res.results[ci]["xout"] for ci in core_ids], axis=0)
    return x16.astype(np.float32).reshape(B, S, D, 2)


# revision 17
# speedup vs baseline: 1.4487x; 1.0897x over previous
"""Cayley soliton propagator — fused single-pass Trainium2 Bass kernel.

Math: the reference runs 20 PCG iterations on (I + i*k*H) x = (I - i*k*H)
rot(psi) per (batch,token) system, where H is a fixed circulant stencil along
D.  H diagonalizes in the DFT basis with eigenvalues lam_f, so in frequency
space A = 1 + i*k*lam_f is DIAGONAL and every CG iterate is elementwise over
frequencies, with only per-system reductions (dot products over f).  The whole
pipeline therefore fuses into ONE device kernel:

  elementwise phase rotation -> forward modified DFT (PE matmul, the
  (1 - i*k*lam) rhs factor folded into the matrix) -> 20 CG iterations in
  frequency space (vector/pool/scalar engines, diagonal ops + per-system
  accumulations) -> inverse DFT (PE) -> interleaved [.., D, 2] output.

No host round-trip, no collectives.  I/O travels as fp16 (the axon tunnel is
~70 MB/s, so halving bytes halves wall time); all device compute is f32.

Sharding: data-parallel over the flattened system axis N=B*S across 8 cores.
"""

import sys

for _p in ("/opt/trn_rl_repo",):
    if _p not in sys.path:
        sys.path.insert(0, _p)

import numpy as np
import concourse.bass as bass
import concourse.tile as tile
from concourse import bacc, mybir
from concourse.bass_utils import run_bass_kernel_spmd
from concourse.masks import make_identity

f32 = mybir.dt.float32
f16 = mybir.dt.float16
OP = mybir.AluOpType
AF = mybir.ActivationFunctionType

# ---- problem constants (hardcoded per contract) ----
B, S, D = 4, 4096, 512
N = B * S                       # 16384 systems
NCORES = 8
NSYS = N // NCORES              # 2048 systems per core
NTILE = NSYS // 128             # 16 sys-tiles of 128 per core
DT = 0.1
KAP = DT / 2.0                  # 0.05
NIT = 20
NUM_SCALES, BASE_SPARSITY = 3, 5
OFFSETS = [(2 ** s) * j for s in range(NUM_SCALES) for j in range(1, BASE_SPARSITY + 1)]
KCH = 4                         # 512/128 chunks


def _host_const(edge_weights, alpha):
    """Constant tensors for the kernel (computed in fp64, shipped as f32)."""
    w = edge_weights.reshape(-1).astype(np.float64)
    f = np.arange(D)
    deg = 2.0 * w.sum()
    lam = deg - sum(w[k] * 2.0 * np.cos(2 * np.pi * OFFSETS[k] * f / D)
                    for k in range(len(w)))
    dk = KAP * deg
    inv_s2 = 1.0 / (1.0 + dk * dk)

    dmat = np.outer(f, f)
    F = np.exp(-2j * np.pi * dmat / D)              # F[f, d], symmetric
    Fp = (1.0 - 1j * KAP * lam)[:, None] * F        # rhs factor folded in
    PT = np.ascontiguousarray(Fp.real.T)            # [d, f] rhs for fwd matmul
    QT = np.ascontiguousarray(Fp.imag.T)
    QTn = np.ascontiguousarray(-Fp.imag.T)
    G = np.exp(2j * np.pi * dmat / D) / D           # inverse DFT [f, d], symmetric
    GR = np.ascontiguousarray(G.real)
    GI = np.ascontiguousarray(G.imag)
    GIn = np.ascontiguousarray(-G.imag)
    kl = (KAP * lam).reshape(1, D)
    aabs = np.abs(alpha.astype(np.float64)).reshape(1, D)
    cc = np.zeros((1, D))
    cc[0, :4] = [inv_s2, -dk * inv_s2, dk * inv_s2, inv_s2 / D]
    c = dict(PT=PT, QT=QT, QTn=QTn, GR=GR, GI=GI, GIn=GIn,
             kl=kl, aabs=aabs, cc=cc)
    return {k: np.ascontiguousarray(v.astype(np.float32)) for k, v in c.items()}


CONST_NAMES = ("PT", "QT", "QTn", "GR", "GI", "GIn", "kl", "aabs", "cc")


# --------------------------------------------------------------- fused kernel
def _build_fused(ntile=NTILE):
    nsys = ntile * 128
    nc = bacc.Bacc()
    pr_d = nc.declare_dram_parameter("pr16", [nsys, D], f16, isOutput=False)
    pi_d = nc.declare_dram_parameter("pi16", [nsys, D], f16, isOutput=False)
    mat_d = {m: nc.declare_dram_parameter(m, [D, D], f32, isOutput=False)
             for m in ("PT", "QT", "QTn", "GR", "GI", "GIn")}
    kl_d = nc.declare_dram_parameter("kl", [1, D], f32, isOutput=False)
    aa_d = nc.declare_dram_parameter("aabs", [1, D], f32, isOutput=False)
    cc_d = nc.declare_dram_parameter("cc", [1, D], f32, isOutput=False)
    x_d = nc.declare_dram_parameter("xout", [nsys, 2 * D], f16, isOutput=True)

    with tile.TileContext(nc) as tc:
        with tc.tile_pool(name="singles", bufs=1) as singles, \
             tc.tile_pool(name="io", bufs=3) as io, \
             tc.tile_pool(name="tmp", bufs=2) as tmp, \
             tc.tile_pool(name="cols", bufs=2) as colsp, \
             tc.tile_pool(name="trT", bufs=2) as trTp, \
             tc.tile_pool(name="bh", bufs=2) as bhp, \
             tc.tile_pool(name="cg", bufs=2) as cgp, \
             tc.tile_pool(name="sc", bufs=2) as scp, \
             tc.tile_pool(name="xt", bufs=2) as xtp, \
             tc.tile_pool(name="outp", bufs=3) as outp, \
             tc.tile_pool(name="pst", bufs=2, space="PSUM") as pst, \
             tc.tile_pool(name="psf", bufs=1, space="PSUM") as psf, \
             tc.tile_pool(name="psx", bufs=1, space="PSUM") as psx:

            # ---- constants into SBUF ----
            mats = {}
            for m in ("PT", "QT", "QTn", "GR", "GI", "GIn"):
                t = singles.tile([128, KCH * D], f32, name=m)
                for k in range(KCH):
                    nc.sync.dma_start(t[:, k * D:(k + 1) * D],
                                      mat_d[m][k * 128:(k + 1) * 128, :])
                mats[m] = t
            KL = singles.tile([128, D], f32, name="KL")
            nc.gpsimd.dma_start(out=KL[:], in_=kl_d[:].to_broadcast([128, D]))
            aab = singles.tile([128, D], f32, name="aab")
            nc.gpsimd.dma_start(out=aab[:], in_=aa_d[:].to_broadcast([128, D]))
            CC = singles.tile([128, D], f32, name="CC")
            nc.gpsimd.dma_start(out=CC[:], in_=cc_d[:].to_broadcast([128, D]))
            ident = singles.tile([128, 128], f32, name="ident")
            make_identity(nc, ident[:])
            nhalfpi = singles.tile([128, 1], f32, name="nhalfpi")
            nc.vector.memset(nhalfpi[:], float(-np.pi / 2))
            c_is2 = CC[:, 0:1]       # inv_s2
            c_nds = CC[:, 1:2]       # -dk*inv_s2
            c_pds = CC[:, 2:3]       # +dk*inv_s2
            c_isd = CC[:, 3:4]       # inv_s2/D

            for t0 in range(ntile):
                rows = slice(t0 * 128, (t0 + 1) * 128)

                # ======== stage A: load + phase rotation ========
                pr16 = io.tile([128, D], f16, tag="pr16")
                pi16 = io.tile([128, D], f16, tag="pi16")
                nc.sync.dma_start(pr16[:], pr_d[rows, :])
                nc.sync.dma_start(pi16[:], pi_d[rows, :])
                prt = tmp.tile([128, D], f32, tag="prt")
                pit = tmp.tile([128, D], f32, tag="pit")
                nc.scalar.copy(prt[:], pr16[:])
                nc.vector.tensor_copy(pit[:], pi16[:])

                cols = colsp.tile([128, 16], f32, tag="cols")
                ta = tmp.tile([128, D], f32, tag="ta")
                tb = tmp.tile([128, D], f32, tag="tb")
                nc.vector.scalar_tensor_tensor(
                    out=ta[:], in0=prt[:], scalar=1.0, in1=prt[:],
                    op0=OP.mult, op1=OP.mult, accum_out=cols[:, 0:1])
                nc.vector.scalar_tensor_tensor(
                    out=tb[:], in0=pit[:], scalar=1.0, in1=pit[:],
                    op0=OP.mult, op1=OP.mult, accum_out=cols[:, 1:2])
                ir = tmp.tile([128, D], f32, tag="ir")
                nc.gpsimd.tensor_tensor(out=ir[:], in0=ta[:], in1=tb[:], op=OP.add)
                # norm_in = c0+c1 ; rm = 1/max(norm_in/D, 1e-6) ; nrm = -rm
                nc.vector.tensor_tensor(out=cols[:, 2:3], in0=cols[:, 0:1],
                                        in1=cols[:, 1:2], op=OP.add)
                nc.vector.tensor_scalar(out=cols[:, 3:4], in0=cols[:, 2:3],
                                        scalar1=1.0 / D, scalar2=1e-6,
                                        op0=OP.mult, op1=OP.max)
                nc.vector.reciprocal(out=cols[:, 4:5], in_=cols[:, 3:4])
                nc.vector.tensor_scalar(out=cols[:, 5:6], in0=cols[:, 4:5],
                                        scalar1=-1.0, scalar2=None, op0=OP.mult)
                # u = exp(-ir*rm); half-angle: cos_p = 1-2*sin^2(pi*u-pi/2) ...
                u = tmp.tile([128, D], f32, tag="u")
                nc.scalar.activation(out=u[:], in_=ir[:], func=AF.Exp,
                                     bias=0.0, scale=cols[:, 5:6])
                shalf = tmp.tile([128, D], f32, tag="ta")
                nc.scalar.activation(out=shalf[:], in_=u[:], func=AF.Sin,
                                     bias=nhalfpi[:], scale=float(np.pi))
                chalf = tmp.tile([128, D], f32, tag="tb")
                nc.scalar.activation(out=chalf[:], in_=u[:], func=AF.Sin,
                                     bias=0.0, scale=float(np.pi))
                q1 = tmp.tile([128, D], f32, tag="u")
                nc.vector.tensor_tensor(out=q1[:], in0=shalf[:], in1=shalf[:], op=OP.mult)
                cp = tmp.tile([128, D], f32, tag="cp")
                nc.vector.tensor_scalar(out=cp[:], in0=q1[:], scalar1=-2.0,
                                        scalar2=1.0, op0=OP.mult, op1=OP.add)
                q2 = tmp.tile([128, D], f32, tag="q2")
                nc.gpsimd.tensor_tensor(out=q2[:], in0=shalf[:], in1=chalf[:], op=OP.mult)
                sp = tmp.tile([128, D], f32, tag="sp")
                nc.vector.tensor_scalar(out=sp[:], in0=q2[:], scalar1=-2.0,
                                        scalar2=None, op0=OP.mult)
                # env = min(1 + aabs*(ir*rm)^2, 10) ; renv = 1/env
                tsq = tmp.tile([128, D], f32, tag="ta")
                nc.scalar.activation(out=tsq[:], in_=ir[:], func=AF.Square,
                                     bias=0.0, scale=cols[:, 4:5])
                env = tmp.tile([128, D], f32, tag="tb")
                nc.vector.scalar_tensor_tensor(
                    out=env[:], in0=tsq[:], scalar=1.0, in1=aab[:],
                    op0=OP.mult, op1=OP.mult)
                nc.vector.tensor_scalar(out=env[:], in0=env[:],
                                        scalar1=1.0, scalar2=10.0,
                                        op0=OP.add, op1=OP.min)
                renv = tmp.tile([128, D], f32, tag="renv")
                nc.vector.reciprocal_approx_fast(out=renv[:], in_=env[:])
                renv2 = tmp.tile([128, D], f32, tag="ta")
                nc.scalar.activation(out=renv2[:], in_=renv[:], func=AF.Square)
                # norm_rot = sum(ir * renv^2) (|rot|^2 == ir pointwise)
                scr = tmp.tile([128, D], f32, tag="tb")
                nc.vector.scalar_tensor_tensor(
                    out=scr[:], in0=ir[:], scalar=1.0, in1=renv2[:],
                    op0=OP.mult, op1=OP.mult, accum_out=cols[:, 6:7])
                # sc = min(sqrt((ni+1e-8)/(nr+1e-8)), 10)
                nc.vector.tensor_scalar(out=cols[:, 7:8], in0=cols[:, 6:7],
                                        scalar1=1e-8, scalar2=None, op0=OP.add)
                nc.vector.reciprocal(out=cols[:, 8:9], in_=cols[:, 7:8])
                nc.vector.tensor_scalar(out=cols[:, 9:10], in0=cols[:, 2:3],
                                        scalar1=1e-8, scalar2=None, op0=OP.add)
                nc.vector.tensor_tensor(out=cols[:, 10:11], in0=cols[:, 8:9],
                                        in1=cols[:, 9:10], op=OP.mult)
                nc.scalar.activation(out=cols[:, 11:12], in_=cols[:, 10:11], func=AF.Sqrt)
                nc.vector.tensor_scalar(out=cols[:, 12:13], in0=cols[:, 11:12],
                                        scalar1=10.0, scalar2=None, op0=OP.min)
                fac = tmp.tile([128, D], f32, tag="u")
                nc.vector.tensor_scalar(out=fac[:], in0=renv[:],
                                        scalar1=cols[:, 12:13], scalar2=None,
                                        op0=OP.mult)
                # rot_r = (pr*cos_p - pi*sin_p)*fac ; rot_i = (pr*sin_p + pi*cos_p)*fac
                t1 = tmp.tile([128, D], f32, tag="ta")
                t2 = tmp.tile([128, D], f32, tag="tb")
                nc.vector.tensor_tensor(out=t1[:], in0=prt[:], in1=cp[:], op=OP.mult)
                nc.gpsimd.tensor_tensor(out=t2[:], in0=pit[:], in1=sp[:], op=OP.mult)
                Rt = tmp.tile([128, D], f32, tag="Rt")
                nc.vector.tensor_tensor(out=Rt[:], in0=t1[:], in1=t2[:], op=OP.subtract)
                t3 = tmp.tile([128, D], f32, tag="ta")
                t4 = tmp.tile([128, D], f32, tag="tb")
                nc.gpsimd.tensor_tensor(out=t3[:], in0=prt[:], in1=sp[:], op=OP.mult)
                nc.vector.tensor_tensor(out=t4[:], in0=pit[:], in1=cp[:], op=OP.mult)
                I2t = tmp.tile([128, D], f32, tag="cp")
                nc.vector.tensor_tensor(out=I2t[:], in0=t3[:], in1=t4[:], op=OP.add)
                rr = tmp.tile([128, D], f32, tag="sp")
                nc.vector.tensor_tensor(out=rr[:], in0=Rt[:], in1=fac[:], op=OP.mult)
                ri = tmp.tile([128, D], f32, tag="q2")
                nc.gpsimd.tensor_tensor(out=ri[:], in0=I2t[:], in1=fac[:], op=OP.mult)

                # ======== stage A2: transpose + forward DFT ========
                rrT = trTp.tile([128, D], f32, tag="rrT")
                riT = trTp.tile([128, D], f32, tag="riT")
                for k in range(KCH):
                    pt = pst.tile([128, 128], f32, tag="pt")
                    nc.tensor.transpose(pt[:], rr[:, k * 128:(k + 1) * 128], ident[:])
                    nc.scalar.copy(rrT[:, k * 128:(k + 1) * 128], pt[:])
                    pt2 = pst.tile([128, 128], f32, tag="pt")
                    nc.tensor.transpose(pt2[:], ri[:, k * 128:(k + 1) * 128], ident[:])
                    nc.vector.tensor_copy(riT[:, k * 128:(k + 1) * 128], pt2[:])

                br = bhp.tile([128, D], f32, tag="br")
                bi = bhp.tile([128, D], f32, tag="bi")
                pbr = psf.tile([128, D], f32, tag="pbr")
                for k in range(KCH):
                    nc.tensor.matmul(pbr[:], rrT[:, k * 128:(k + 1) * 128],
                                     mats["PT"][:, k * D:(k + 1) * D],
                                     start=(k == 0), stop=False)
                for k in range(KCH):
                    nc.tensor.matmul(pbr[:], riT[:, k * 128:(k + 1) * 128],
                                     mats["QTn"][:, k * D:(k + 1) * D],
                                     start=False, stop=(k == KCH - 1))
                nc.scalar.copy(br[:], pbr[:])
                pbi = psf.tile([128, D], f32, tag="pbi")
                for k in range(KCH):
                    nc.tensor.matmul(pbi[:], rrT[:, k * 128:(k + 1) * 128],
                                     mats["QT"][:, k * D:(k + 1) * D],
                                     start=(k == 0), stop=False)
                for k in range(KCH):
                    nc.tensor.matmul(pbi[:], riT[:, k * 128:(k + 1) * 128],
                                     mats["PT"][:, k * D:(k + 1) * D],
                                     start=False, stop=(k == KCH - 1))
                nc.vector.tensor_copy(bi[:], pbi[:])

                # ======== stage B: CG in frequency space ========
                # planes: p_r, p_i, xh_r, xh_i, Apr, Api, w1, w2; r lives in br/bi
                p_r = cgp.tile([128, D], f32, tag="p_r")
                p_i = cgp.tile([128, D], f32, tag="p_i")
                xhr = cgp.tile([128, D], f32, tag="xhr")
                xhi = cgp.tile([128, D], f32, tag="xhi")
                Apr = cgp.tile([128, D], f32, tag="Apr")
                Api = cgp.tile([128, D], f32, tag="Api")
                w1 = cgp.tile([128, D], f32, tag="w1")
                w2 = cgp.tile([128, D], f32, tag="w2")
                sc = scp.tile([128, 288], f32, tag="sc")

                # engine rules: Pool = tensor_tensor only; vector = reductions
                # + small [128,1] algebra; scalar(ACT) = per-system Copy-scale
                # broadcasts (tableless).
                nc.vector.memset(xhr[:], 0.0)
                nc.vector.memset(xhi[:], 0.0)
                # rz0 = (inv_s2/D) * sum(|bhat|^2) ; p0 = m * bhat
                nc.vector.scalar_tensor_tensor(
                    out=w1[:], in0=br[:], scalar=1.0, in1=br[:],
                    op0=OP.mult, op1=OP.mult, accum_out=sc[:, 0:1])
                nc.vector.scalar_tensor_tensor(
                    out=w2[:], in0=bi[:], scalar=1.0, in1=bi[:],
                    op0=OP.mult, op1=OP.mult, accum_out=sc[:, 1:2])
                nc.gpsimd.tensor_tensor(out=sc[:, 2:3], in0=sc[:, 0:1],
                                        in1=sc[:, 1:2], op=OP.add)
                nc.vector.tensor_scalar(out=sc[:, 3:4], in0=sc[:, 2:3],
                                        scalar1=c_isd, scalar2=None, op0=OP.mult)
                nc.scalar.activation(out=p_r[:], in_=br[:], func=AF.Copy, scale=c_is2)
                nc.scalar.activation(out=Api[:], in_=bi[:], func=AF.Copy, scale=c_nds)
                nc.gpsimd.tensor_tensor(out=p_r[:], in0=p_r[:], in1=Api[:], op=OP.add)
                nc.scalar.activation(out=p_i[:], in_=bi[:], func=AF.Copy, scale=c_is2)
                nc.scalar.activation(out=Apr[:], in_=br[:], func=AF.Copy, scale=c_pds)
                nc.gpsimd.tensor_tensor(out=p_i[:], in0=p_i[:], in1=Apr[:], op=OP.add)

                rz = sc[:, 3:4]
                for it in range(NIT):
                    cb = 4 + it * 14
                    def col(j, cb=cb):
                        return sc[:, cb + j:cb + j + 1]
                    # Ap = (1 + i*kl) p
                    nc.gpsimd.tensor_tensor(out=w1[:], in0=KL[:], in1=p_i[:], op=OP.mult)
                    nc.gpsimd.tensor_tensor(out=Apr[:], in0=p_r[:], in1=w1[:], op=OP.subtract)
                    nc.gpsimd.tensor_tensor(out=w2[:], in0=KL[:], in1=p_r[:], op=OP.mult)
                    nc.gpsimd.tensor_tensor(out=Api[:], in0=p_i[:], in1=w2[:], op=OP.add)
                    # pAp = (1/D)*(sum(p_r*Apr) + sum(p_i*Api)) ; a = rz/(pAp+eps)
                    nc.vector.scalar_tensor_tensor(
                        out=w1[:], in0=p_r[:], scalar=1.0 / D, in1=Apr[:],
                        op0=OP.mult, op1=OP.mult, accum_out=col(0))
                    nc.vector.scalar_tensor_tensor(
                        out=w2[:], in0=p_i[:], scalar=1.0 / D, in1=Api[:],
                        op0=OP.mult, op1=OP.mult, accum_out=col(1))
                    nc.vector.scalar_tensor_tensor(
                        out=col(3), in0=col(0), scalar=1e-30, in1=col(1),
                        op0=OP.add, op1=OP.add)
                    nc.vector.reciprocal(out=col(4), in_=col(3))
                    nc.gpsimd.tensor_tensor(out=col(5), in0=rz, in1=col(4), op=OP.mult)  # a
                    nc.scalar.activation(out=col(6), in_=col(5), func=AF.Copy,
                                         scale=-1.0)                                     # -a
                    # x += a*p ; r -= a*Ap  (scalar engine broadcasts, Pool adds)
                    nc.scalar.activation(out=w1[:], in_=p_r[:], func=AF.Copy, scale=col(5))
                    nc.gpsimd.tensor_tensor(out=xhr[:], in0=xhr[:], in1=w1[:], op=OP.add)
                    nc.scalar.activation(out=w2[:], in_=p_i[:], func=AF.Copy, scale=col(5))
                    nc.gpsimd.tensor_tensor(out=xhi[:], in0=xhi[:], in1=w2[:], op=OP.add)
                    nc.scalar.activation(out=Apr[:], in_=Apr[:], func=AF.Copy, scale=col(6))
                    nc.gpsimd.tensor_tensor(out=br[:], in0=br[:], in1=Apr[:], op=OP.add)
                    nc.scalar.activation(out=Api[:], in_=Api[:], func=AF.Copy, scale=col(6))
                    nc.gpsimd.tensor_tensor(out=bi[:], in0=bi[:], in1=Api[:], op=OP.add)
                    # rz_new = (inv_s2/D) * ||r||^2 ; beta = rz_new/(rz+eps)
                    nc.vector.scalar_tensor_tensor(
                        out=w1[:], in0=br[:], scalar=1.0, in1=br[:],
                        op0=OP.mult, op1=OP.mult, accum_out=col(7))
                    nc.vector.scalar_tensor_tensor(
                        out=w2[:], in0=bi[:], scalar=1.0, in1=bi[:],
                        op0=OP.mult, op1=OP.mult, accum_out=col(8))
                    nc.gpsimd.tensor_tensor(out=col(9), in0=col(7), in1=col(8), op=OP.add)
                    nc.vector.tensor_scalar(out=col(10), in0=col(9),
                                            scalar1=c_isd, scalar2=None, op0=OP.mult)
                    nc.vector.tensor_scalar(out=col(11), in0=rz,
                                            scalar1=1e-30, scalar2=None, op0=OP.add)
                    nc.vector.reciprocal(out=col(12), in_=col(11))
                    nc.gpsimd.tensor_tensor(out=col(13), in0=col(10), in1=col(12),
                                            op=OP.mult)  # beta
                    # p = m*r + beta*p
                    nc.scalar.activation(out=Apr[:], in_=br[:], func=AF.Copy, scale=c_is2)
                    nc.scalar.activation(out=Api[:], in_=bi[:], func=AF.Copy, scale=c_nds)
                    nc.gpsimd.tensor_tensor(out=Apr[:], in0=Apr[:], in1=Api[:], op=OP.add)
                    nc.scalar.activation(out=w1[:], in_=p_r[:], func=AF.Copy, scale=col(13))
                    nc.gpsimd.tensor_tensor(out=p_r[:], in0=Apr[:], in1=w1[:], op=OP.add)
                    nc.scalar.activation(out=Apr[:], in_=bi[:], func=AF.Copy, scale=c_is2)
                    nc.scalar.activation(out=Api[:], in_=br[:], func=AF.Copy, scale=c_pds)
                    nc.gpsimd.tensor_tensor(out=Apr[:], in0=Apr[:], in1=Api[:], op=OP.add)
                    nc.scalar.activation(out=w2[:], in_=p_i[:], func=AF.Copy, scale=col(13))
                    nc.gpsimd.tensor_tensor(out=p_i[:], in0=Apr[:], in1=w2[:], op=OP.add)
                    rz = col(10)

                # ======== stage C: inverse DFT + interleave + store ========
                xrT = xtp.tile([128, D], f32, tag="xrT")
                xiT = xtp.tile([128, D], f32, tag="xiT")
                for k in range(KCH):
                    pt = pst.tile([128, 128], f32, tag="pt")
                    nc.tensor.transpose(pt[:], xhr[:, k * 128:(k + 1) * 128], ident[:])
                    nc.scalar.copy(xrT[:, k * 128:(k + 1) * 128], pt[:])
                    pt2 = pst.tile([128, 128], f32, tag="pt")
                    nc.tensor.transpose(pt2[:], xhi[:, k * 128:(k + 1) * 128], ident[:])
                    nc.vector.tensor_copy(xiT[:, k * 128:(k + 1) * 128], pt2[:])

                pxr = psx.tile([128, D], f32, tag="pxr")
                for k in range(KCH):
                    nc.tensor.matmul(pxr[:], xrT[:, k * 128:(k + 1) * 128],
                                     mats["GR"][:, k * D:(k + 1) * D],
                                     start=(k == 0), stop=False)
                for k in range(KCH):
                    nc.tensor.matmul(pxr[:], xiT[:, k * 128:(k + 1) * 128],
                                     mats["GIn"][:, k * D:(k + 1) * D],
                                     start=False, stop=(k == KCH - 1))
                pxi = psx.tile([128, D], f32, tag="pxi")
                for k in range(KCH):
                    nc.tensor.matmul(pxi[:], xrT[:, k * 128:(k + 1) * 128],
                                     mats["GI"][:, k * D:(k + 1) * D],
                                     start=(k == 0), stop=False)
                for k in range(KCH):
                    nc.tensor.matmul(pxi[:], xiT[:, k * 128:(k + 1) * 128],
                                     mats["GR"][:, k * D:(k + 1) * D],
                                     start=False, stop=(k == KCH - 1))
                ot = outp.tile([128, 2 * D], f16, tag="ot")
                ov = ot[:].rearrange("p (d t) -> p d t", t=2)
                nc.scalar.copy(ov[:, :, 0], pxr[:])
                nc.vector.tensor_copy(ov[:, :, 1], pxi[:])
                nc.sync.dma_start(x_d[rows, :], ot[:])
    nc.compile()
    return nc


_cache = {}


def _make_exec(nc, replicated=()):
    """Sharded jit runner for a Bass module; global arrays in/out.

    Mirrors bass2jax.run_bass_via_pjrt's multi-core path but keeps the jitted
    callable so constants can stay device-resident between calls, and takes
    pre-sharded donated zero output buffers (cheap, created on-device)."""
    import jax
    from jax.sharding import Mesh, PartitionSpec, NamedSharding
    from concourse import bass2jax, mybir as _mb

    bass2jax.install_neuronx_cc_hook()
    partition_name = (nc.partition_id_tensor.name
                      if nc.partition_id_tensor else None)
    in_names, out_names, out_avals, out_shapes = [], [], [], []
    for alloc in nc.m.functions[0].allocations:
        if not isinstance(alloc, _mb.MemoryLocationSet):
            continue
        name = alloc.memorylocations[0].name
        if alloc.kind == "ExternalInput":
            if name != partition_name:
                in_names.append(name)
        elif alloc.kind == "ExternalOutput":
            out_names.append(name)
            shape = tuple(alloc.tensor_shape)
            dtype = _mb.dt.np(alloc.dtype)
            out_avals.append(jax.core.ShapedArray(shape, dtype))
            out_shapes.append(((NCORES * shape[0],) + shape[1:], dtype))
    n_params = len(in_names)
    all_in = list(in_names) + list(out_names)
    if partition_name is not None:
        all_in.append(partition_name)

    def _body(*args):
        operands = list(args)
        if partition_name is not None:
            operands.append(bass2jax.partition_id_tensor())
        return tuple(bass2jax._bass_exec_p.bind(
            *operands,
            out_avals=tuple(out_avals),
            in_names=tuple(all_in),
            out_names=tuple(out_names),
            lowering_input_output_aliases=(),
            sim_require_finite=True,
            sim_require_nnan=True,
            nc=nc,
        ))

    devices = jax.devices()[:NCORES]
    mesh = Mesh(np.asarray(devices), ("core",))
    n_outs = len(out_names)
    from jax.experimental.shard_map import shard_map
    in_specs = tuple(
        PartitionSpec() if nm in replicated else PartitionSpec("core")
        for nm in in_names
    ) + (PartitionSpec("core"),) * n_outs
    sharded = jax.jit(
        shard_map(_body, mesh=mesh,
                  in_specs=in_specs,
                  out_specs=(PartitionSpec("core"),) * n_outs,
                  check_rep=False),
        keep_unused=True,
    )

    # The output operands are an ABI placeholder: the NEFF binds outputs to
    # the custom-call RESULTS (fresh buffers), and this kernel writes every
    # output element, so the placeholder content never matters.  Create it
    # once on-device and reuse (no donation), avoiding a per-call dispatch.
    import jax.numpy as jnp
    zs = [
        jax.jit(lambda shp=shp, dt=dt: jnp.zeros(shp, dt),
                out_shardings=NamedSharding(mesh, PartitionSpec("core")))()
        for shp, dt in out_shapes
    ]
    for z in zs:
        z.block_until_ready()

    def run(feed):  # feed: dict name -> global array (np or jax)
        args = [feed[n] for n in in_names]
        return sharded(*args, *zs)

    return run, out_names, mesh


def kernel(psi_r, psi_i, alpha, edge_weights):
    psi_r = np.asarray(psi_r, np.float32).reshape(N, D)
    psi_i = np.asarray(psi_i, np.float32).reshape(N, D)
    alpha = np.asarray(alpha, np.float64)
    edge_weights = np.asarray(edge_weights, np.float64)
    try:
        return _kernel_fast(psi_r, psi_i, alpha, edge_weights)
    except Exception:
        return _kernel_safe(psi_r, psi_i, alpha, edge_weights)


def _get_consts_dev(alpha, edge_weights, mesh):
    """Device-resident replicated constant tensors, cached by content."""
    import jax
    from jax.sharding import NamedSharding, PartitionSpec
    key = (edge_weights.tobytes(), alpha.tobytes())
    ent = _cache.get("consts")
    if ent is not None and ent[0] == key:
        return ent[1]
    c = _host_const(edge_weights, alpha)
    rep = NamedSharding(mesh, PartitionSpec())
    dev = {k: jax.device_put(c[k], rep) for k in CONST_NAMES}
    _cache["consts"] = (key, dev)
    return dev


def _pool():
    import concurrent.futures as cf
    if "pool" not in _cache:
        _cache["pool"] = cf.ThreadPoolExecutor(16)
    return _cache["pool"]


def _kernel_fast(psi_r, psi_i, alpha, edge_weights):
    import jax
    from jax.sharding import NamedSharding, PartitionSpec
    if "fused" not in _cache:
        _cache["fused"] = _build_fused(ntile=NTILE)
    if "ex" not in _cache:
        _cache["ex"] = _make_exec(_cache["fused"], replicated=CONST_NAMES)
    run, out_names, mesh = _cache["ex"]
    consts = _get_consts_dev(alpha, edge_weights, mesh)
    pool = _pool()
    sh = NamedSharding(mesh, PartitionSpec("core"))

    # chunk-parallel cast of psi_r first so its upload starts immediately;
    # psi_i casts while psi_r streams, then queues behind it on the tunnel.
    # Dispatch right after so the dispatch RPC also overlaps the transfers.
    def cast16(x):
        o = np.empty(x.shape, np.float16)
        rows = x.shape[0] // 8

        def w(i):
            np.copyto(o[i * rows:(i + 1) * rows], x[i * rows:(i + 1) * rows])
        list(pool.map(w, range(8)))
        return o
    da = jax.device_put(cast16(psi_r), sh)
    db = jax.device_put(cast16(psi_i), sh)

    feed = dict(consts)
    feed["pr16"] = da
    feed["pi16"] = db
    h = dict(zip(out_names, run(feed)))["xout"]

    # parallel per-shard fetch (pure f16 pull in the pool threads); the main
    # thread upcasts each part as it lands, overlapping cast with fetch
    import queue as _q
    shards = h.addressable_shards
    for s in shards:
        s.data.copy_to_host_async()
    doneq = _q.Queue()

    def fetch(s):
        doneq.put((s.index[0].start or 0, np.asarray(s.data)))
    for s in shards:
        pool.submit(fetch, s)
    final = np.empty((N, 2 * D), np.float32)
    for _ in range(len(shards)):
        i0, a = doneq.get()
        final[i0:i0 + a.shape[0]] = a
    return final.reshape(B, S, D, 2)


def _kernel_safe(psi_r, psi_i, alpha, edge_weights):
    if "fused" not in _cache:
        _cache["fused"] = _build_fused(ntile=NTILE)
    knl = _cache["fused"]
    c = _host_const(edge_weights, alpha)
    pr16 = psi_r.astype(np.float16)
    pi16 = psi_i.astype(np.float16)
    core_ids = list(range(NCORES))
    ins = []
    for ci in core_ids:
        rows = slice(ci * NSYS, (ci + 1) * NSYS)
        d = {k: c[k] for k in CONST_NAMES}
        d["pr16"] = pr16[rows]
        d["pi16"] = pi16[rows]
        ins.append(d)
    res = run_bass_kernel_spmd(knl, ins, core_ids)
    x16 = np.concatenate([res.results[ci]["xout"] for ci in core_ids], axis=0)
    return x16.astype(np.float32).reshape(B, S, D, 2)
